# revision 4
# baseline (speedup 1.0000x reference)
"""Trainium2 Bass kernel for nn_KnowldgeShifter (moe_routing).

Computes, for batch N=32, experts K=32, tokens T=128, hidden H=768:
  score = (pe1 @ W_k.T + b_k) . (concat(ctx[:,2], tracked) @ W_cqk.T + b_cqk)
  masked by ck_mask, plus per-row top-1 (label) gathers of
  pool_encoded_{0,1}, pool_mask, pool_tokens.

Sharding: data-parallel over batch N across 8 NeuronCores (4 rows/core).
The key memory optimization: pool_encoded_0 (403MB) is never read in
full -- each core gathers only its label-selected [T,H] blocks via
indirect DMA (offsets computed on-device from the label input).

Algebraic restructure to keep every matmul transpose-free on device:
  score[n,k] = pe1_ext[n,k,:] . u_ext[n,:]
  u_ext = A_ext @ [W_k | b_k]          (contract over g, W_k natural)
  A     = cqk_ext @ [W_cqk.T; b_cqk]   (contract over j, W_cqk.T host-prepped)
where _ext appends a ones column so biases ride along in the matmul.
"""

import numpy as np

import concourse.bacc as bacc
import concourse.bass as bass
import concourse.mybir as mybir
import concourse.tile as tile
from concourse.bass import IndirectOffsetOnAxis
from concourse.bass_utils import run_bass_kernel_spmd

# Problem shapes (hardcoded per contract).
N, K, T, H = 32, 32, 128, 768
NCORES = 8
R = N // NCORES            # batch rows per core = 4
JT = 13                    # contraction tiles for stage 1 (2H+1 -> 13*128)
JPAD = JT * 128            # 1664
ST = H // 128              # contraction tiles for stage 2 = 6
HP = H + 1                 # 769 (ones column appended)
NEGINF = -1e20

f32 = mybir.dt.float32
i32 = mybir.dt.int32
u8 = mybir.dt.uint8

_CACHE = {}


def _build_nc(tokc):
    """Build the (label-independent) SPMD Tile program. tokc = int32 columns
    per token row (T for int32 tokens, 2T for int64)."""
    nc = bacc.Bacc(
        "TRN2",
        target_bir_lowering=False,
        debug=False,
        enable_asserts=False,
        num_devices=NCORES,
    )

    # Inputs (per-core shards; weights/constants replicated).
    d_cqkT = nc.dram_tensor("cqkT", [128, JT * R], f32, kind="ExternalInput")
    d_wcqkT = nc.dram_tensor("wcqkT", [JPAD, H], f32, kind="ExternalInput")
    d_wk = nc.dram_tensor("wk", [H, HP], f32, kind="ExternalInput")
    d_pe1 = nc.dram_tensor("pe1", [R * K, H], f32, kind="ExternalInput")
    d_pe0 = nc.dram_tensor("pe0", [R * K * 32, T * H // 32], f32, kind="ExternalInput")
    d_ptok = nc.dram_tensor("ptok", [R * K, tokc], i32, kind="ExternalInput")
    d_pmask = nc.dram_tensor("pmask", [R * K, T], u8, kind="ExternalInput")
    d_ck = nc.dram_tensor("ckmask", [R * K, 1], u8, kind="ExternalInput")
    d_label = nc.dram_tensor("label", [R, 1], i32, kind="ExternalInput")
    d_bt = nc.dram_tensor("bt", [R, 128], f32, kind="ExternalInput")
    d_c2 = nc.dram_tensor("c2", [R, 1], f32, kind="ExternalInput")
    d_c3 = nc.dram_tensor("c3", [128, 1], f32, kind="ExternalInput")
    d_id = nc.dram_tensor("ident", [128, 128], f32, kind="ExternalInput")

    # Outputs.
    o_score = nc.dram_tensor("score", [R * K, 1], f32, kind="ExternalOutput")
    o_senc = nc.dram_tensor("senc", [R * T * H // 3072, T * H // 32], f32,
                            kind="ExternalOutput")
    o_smask = nc.dram_tensor("smask", [R, T], u8, kind="ExternalOutput")
    o_suse = nc.dram_tensor("suse", [R, H], f32, kind="ExternalOutput")
    o_sidx = nc.dram_tensor("sidx", [R, tokc], i32, kind="ExternalOutput")

    add = mybir.AluOpType.add
    mult = mybir.AluOpType.mult

    with tile.TileContext(nc) as tc:
        with (
            tc.tile_pool(name="sb", bufs=1) as sb,
            tc.tile_pool(name="ps", bufs=1, space="PSUM") as ps,
        ):
            # ---- small constants / shards
            cqkT_sb = sb.tile([128, JT * R], f32)
            nc.sync.dma_start(cqkT_sb[:], d_cqkT.ap())
            bt_sb = sb.tile([R, 128], f32)
            nc.sync.dma_start(bt_sb[:], d_bt.ap())
            c2_sb = sb.tile([R, 1], f32)
            nc.sync.dma_start(c2_sb[:], d_c2.ap())
            c3_sb = sb.tile([128, 1], f32)
            nc.sync.dma_start(c3_sb[:], d_c3.ap())
            id_sb = sb.tile([128, 128], f32)
            nc.sync.dma_start(id_sb[:], d_id.ap())
            lbl_sb = sb.tile([R, 1], i32)
            nc.sync.dma_start(lbl_sb[:], d_label.ap())
            ck_sb = sb.tile([R * K, 1], u8)
            nc.sync.dma_start(ck_sb[:], d_ck.ap())
            pe1_sb = sb.tile([R * K, HP], f32)
            nc.sync.dma_start(pe1_sb[:, 0:H], d_pe1.ap())
            nc.vector.memset(pe1_sb[:, H:HP], 1.0)

            # ---- gather index computation (from label input, on device)
            lblf = sb.tile([R, 1], f32)
            nc.vector.tensor_copy(lblf[:], lbl_sb[:])
            # idx2[r] = r*K + label[r]  (row index into [R*K, ...] views)
            idx2f = sb.tile([R, 1], f32)
            nc.vector.tensor_tensor(out=idx2f[:], in0=lblf[:], in1=c2_sb[:], op=add)
            idx2 = sb.tile([R, 1], i32)
            nc.vector.tensor_copy(idx2[:], idx2f[:])
            # idx3[p] = (p//32)*1024 + label[p//32]*32 + p%32
            #   (row index into the [4096, 3072] view of pe0)
            p_lbl = ps.tile([128, 1], f32, tag="tiny", bufs=2)
            nc.tensor.matmul(p_lbl[:], lhsT=bt_sb[:], rhs=lblf[:],
                             start=True, stop=True)
            idx3f = sb.tile([128, 1], f32)
            nc.vector.tensor_scalar(out=idx3f[:], in0=p_lbl[:], scalar1=32.0,
                                    scalar2=None, op0=mult)
            nc.vector.tensor_tensor(out=idx3f[:], in0=idx3f[:], in1=c3_sb[:], op=add)
            idx3 = sb.tile([128, 1], i32)
            nc.vector.tensor_copy(idx3[:], idx3f[:])

            # ---- indirect gathers (the memory-critical path)
            g0 = sb.tile([128, T * H // 32], f32)
            nc.gpsimd.indirect_dma_start(
                out=g0[:], out_offset=None, in_=d_pe0.ap(),
                in_offset=IndirectOffsetOnAxis(ap=idx3[:, :1], axis=0))
            nc.sync.dma_start(o_senc.ap(), g0[:])
            g1 = sb.tile([R, H], f32)
            nc.gpsimd.indirect_dma_start(
                out=g1[:], out_offset=None, in_=d_pe1.ap(),
                in_offset=IndirectOffsetOnAxis(ap=idx2[:, :1], axis=0))
            nc.sync.dma_start(o_suse.ap(), g1[:])
            g2 = sb.tile([R, T], u8)
            nc.gpsimd.indirect_dma_start(
                out=g2[:], out_offset=None, in_=d_pmask.ap(),
                in_offset=IndirectOffsetOnAxis(ap=idx2[:, :1], axis=0))
            nc.sync.dma_start(o_smask.ap(), g2[:])
            g3 = sb.tile([R, tokc], i32)
            nc.gpsimd.indirect_dma_start(
                out=g3[:], out_offset=None, in_=d_ptok.ap(),
                in_offset=IndirectOffsetOnAxis(ap=idx2[:, :1], axis=0))
            nc.sync.dma_start(o_sidx.ap(), g3[:])

            # ---- stage 1: A[r,g] = sum_j cqk_ext[r,j] * wcqkT[j,g]
            wc_tiles = []
            for t in range(JT):
                w_t = sb.tile([128, H], f32, tag=f"wc{t}")
                nc.sync.dma_start(w_t[:], d_wcqkT.ap()[t * 128:(t + 1) * 128, :])
                wc_tiles.append(w_t)
            pA = ps.tile([R, H], f32, tag="acc")
            for lo, hi in ((0, 512), (512, H)):
                for t in range(JT):
                    nc.tensor.matmul(pA[:, lo:hi],
                                     lhsT=cqkT_sb[:, t * R:(t + 1) * R],
                                     rhs=wc_tiles[t][:, lo:hi],
                                     start=(t == 0), stop=(t == JT - 1))
            A_sb = sb.tile([R, H], f32)
            nc.vector.tensor_copy(A_sb[:], pA[:])

            # ---- transpose A -> AT (PE transpose, 128-col blocks)
            AT_sb = sb.tile([128, ST * R], f32)
            for s in range(ST):
                pT = ps.tile([128, R], f32, tag="tiny", bufs=2)
                nc.tensor.transpose(pT[:], A_sb[:, s * 128:(s + 1) * 128],
                                    id_sb[:R, :R])
                nc.vector.tensor_copy(AT_sb[:, s * R:(s + 1) * R], pT[:])

            # ---- stage 2: u_ext[r,h] = sum_g A[r,g] * wk_ext[g,h]
            wk_tiles = []
            for s in range(ST):
                wk_s = sb.tile([128, HP], f32, tag=f"wk{s}")
                nc.sync.dma_start(wk_s[:], d_wk.ap()[s * 128:(s + 1) * 128, :])
                wk_tiles.append(wk_s)
            pU = ps.tile([R, HP], f32, tag="acc")
            for lo, hi in ((0, 512), (512, HP)):
                for s in range(ST):
                    nc.tensor.matmul(pU[:, lo:hi],
                                     lhsT=AT_sb[:, s * R:(s + 1) * R],
                                     rhs=wk_tiles[s][:, lo:hi],
                                     start=(s == 0), stop=(s == ST - 1))
            u_sb = sb.tile([R, HP], f32)
            nc.vector.tensor_copy(u_sb[:], pU[:])

            # ---- broadcast u rows to 32 partitions each: uexp[p,:] = u[p//32,:]
            pUE = ps.tile([128, HP], f32, tag="uexp")
            for lo, hi in ((0, 512), (512, HP)):
                nc.tensor.matmul(pUE[:, lo:hi], lhsT=bt_sb[:], rhs=u_sb[:, lo:hi],
                                 start=True, stop=True)

            # ---- score: rowwise dot + ck_mask select
            prod = sb.tile([R * K, HP], f32)
            nc.vector.tensor_mul(prod[:], pe1_sb[:], pUE[:])
            sc = sb.tile([R * K, 1], f32)
            nc.vector.tensor_reduce(sc[:], prod[:], mybir.AxisListType.X, add)
            negt = sb.tile([R * K, 1], f32)
            nc.vector.memset(negt[:], NEGINF)
            scm = sb.tile([R * K, 1], f32)
            nc.vector.select(scm[:], ck_sb[:], sc[:], negt[:])
            nc.sync.dma_start(o_score.ap(), scm[:])

    nc.compile()
    return nc


def _host_prep(inputs):
    ctx = np.ascontiguousarray(np.asarray(inputs["contexts_encoded_1"], np.float32))
    tku = np.ascontiguousarray(np.asarray(inputs["tracked_knowledge_use"], np.float32))
    pe0 = np.ascontiguousarray(np.asarray(inputs["pool_encoded_0"], np.float32))
    pe1 = np.ascontiguousarray(np.asarray(inputs["pool_encoded_1"], np.float32))
    pm = np.ascontiguousarray(np.asarray(inputs["pool_mask"]))
    ckm = np.ascontiguousarray(np.asarray(inputs["ck_mask"]))
    lbl = np.asarray(inputs["label"]).astype(np.int32)
    ptok = np.ascontiguousarray(np.asarray(inputs["pool_tokens"]))
    Wc = np.asarray(inputs["W_cqk"], np.float32)
    bc = np.asarray(inputs["b_cqk"], np.float32)
    Wk = np.asarray(inputs["W_k"], np.float32)
    bk = np.asarray(inputs["b_k"], np.float32)

    tok_itemsize = ptok.dtype.itemsize
    assert tok_itemsize % 4 == 0
    tokc = T * (tok_itemsize // 4)

    wcqkT = np.zeros((JPAD, H), np.float32)
    wcqkT[:2 * H] = Wc.T
    wcqkT[2 * H] = bc
    wk_ext = np.ascontiguousarray(
        np.concatenate([Wk, bk[:, None]], axis=1).astype(np.float32))

    p = np.arange(128)
    bt = np.zeros((R, 128), np.float32)
    bt[p // K, p] = 1.0
    c2 = (np.arange(R, dtype=np.float32) * K).reshape(R, 1)
    c3 = ((p // K) * (K * 32) + (p % 32)).astype(np.float32).reshape(128, 1)
    ident = np.eye(128, dtype=np.float32)

    in_maps = []
    for c in range(NCORES):
        sl = slice(c * R, (c + 1) * R)
        cqk_ext = np.zeros((R, JPAD), np.float32)
        cqk_ext[:, :H] = ctx[sl, 2, :]
        cqk_ext[:, H:2 * H] = tku[sl]
        cqk_ext[:, 2 * H] = 1.0
        cqkT_sw = np.ascontiguousarray(
            cqk_ext.reshape(R, JT, 128).transpose(2, 1, 0)).reshape(128, JT * R)
        in_maps.append({
            "cqkT": cqkT_sw,
            "wcqkT": wcqkT,
            "wk": wk_ext,
            "pe1": np.ascontiguousarray(pe1[sl]).reshape(R * K, H),
            "pe0": np.ascontiguousarray(pe0[sl]).reshape(R * K * 32, T * H // 32),
            "ptok": np.ascontiguousarray(ptok[sl]).view(np.int32).reshape(R * K, tokc),
            "pmask": pm[sl].reshape(R * K, T).view(np.uint8),
            "ckmask": ckm[sl].reshape(R * K, 1).view(np.uint8),
            "label": np.ascontiguousarray(lbl[sl]).reshape(R, 1),
            "bt": bt,
            "c2": c2,
            "c3": c3,
            "ident": ident,
        })
    return in_maps, tokc, ptok.dtype, np.asarray(inputs["pool_mask"]).dtype


def _run(inputs, trace=False, trace_cores=None):
    in_maps, tokc, tok_dtype, mask_dtype = _host_prep(inputs)
    if tokc not in _CACHE:
        _CACHE[tokc] = _build_nc(tokc)
    nc = _CACHE[tokc]
    res = run_bass_kernel_spmd(
        nc, in_maps, core_ids=list(range(NCORES)),
        trace=trace, trace_cores=trace_cores,
    )
    score = np.concatenate(
        [res.results[c]["score"].reshape(R, K) for c in range(NCORES)], axis=0)
    senc = np.concatenate(
        [res.results[c]["senc"].reshape(R, T, H) for c in range(NCORES)], axis=0)
    smask = np.concatenate(
        [res.results[c]["smask"].reshape(R, T) for c in range(NCORES)],
        axis=0).view(mask_dtype)
    suse = np.concatenate(
        [res.results[c]["suse"].reshape(R, H) for c in range(NCORES)], axis=0)
    sidx = np.concatenate(
        [res.results[c]["sidx"].reshape(R, tokc) for c in range(NCORES)],
        axis=0).view(tok_dtype).reshape(N, T)
    return (score, senc, smask, suse, sidx), res


def kernel(**inputs):
    outs, _ = _run(inputs, trace=False)
    return outs


# revision 7
# speedup vs baseline: 1.7504x; 1.7504x over previous
"""Trainium2 Bass kernel for nn_KnowldgeShifter (moe_routing).

Computes, for batch N=32, experts K=32, tokens T=128, hidden H=768:
  score = (pe1 @ W_k.T + b_k) . (concat(ctx[:,2], tracked) @ W_cqk.T + b_cqk)
  masked by ck_mask, plus per-row top-1 (label) gathers of
  pool_encoded_{0,1}, pool_mask, pool_tokens.

Sharding: data-parallel over batch N across 8 NeuronCores (4 rows/core).
Key memory optimization: pool_encoded_0 (403MB) is never read in full --
each core gathers only its label-selected [T,H] blocks via indirect DMA
(offsets computed on-device from the label input). Gather outputs are
bit-exact f32 copies.

Matmuls run in bf16 (operands) with f32 PSUM accumulation; measured
score error ~3e-3 relative. Algebraic restructure keeps every matmul
transpose-free on device:
  score[n,k] = pe1_ext[n,k,:] . u_ext[n,:]
  u_ext = A_ext @ [W_k | b_k]          (contract over g, W_k natural)
  A     = cqk_ext @ [W_cqk.T; b_cqk]   (contract over j, W_cqk.T host-prepped)
where _ext appends a ones column so biases ride along in the matmul.
"""

import ml_dtypes
import numpy as np

import concourse.bacc as bacc
import concourse.mybir as mybir
import concourse.tile as tile
from concourse.bass import IndirectOffsetOnAxis
from concourse.bass_utils import run_bass_kernel_spmd

# Problem shapes (hardcoded per contract).
N, K, T, H = 32, 32, 128, 768
NCORES = 8
R = N // NCORES            # batch rows per core = 4
JT = 13                    # contraction tiles for stage 1 (2H+1 -> 13*128)
JPAD = JT * 128            # 1664
ST = H // 128              # contraction tiles for stage 2 = 6
HP = H + 1                 # 769 (ones column appended)
GROWS = 4096               # pe0 viewed as [4096, 3072] rows per core
GCOLS = T * H // 32        # 3072
NEGINF = -1e20
WC_CHUNKS = (5, 4, 4)      # j-tiles per stage-1 weight DMA chunk
WK_CHUNKS = (3, 3)         # g-tiles per stage-2 weight DMA chunk

f32 = mybir.dt.float32
bf16 = mybir.dt.bfloat16
i32 = mybir.dt.int32
u8 = mybir.dt.uint8
bfnp = ml_dtypes.bfloat16

_CACHE = {}


def _build_nc(tokc):
    """Build the (label-independent) SPMD Tile program. tokc = int32 columns
    per token row (T for int32 tokens, 2T for int64)."""
    nc = bacc.Bacc(
        "TRN2",
        target_bir_lowering=False,
        debug=False,
        enable_asserts=False,
        num_devices=NCORES,
    )

    # Inputs (per-core shards; weights/constants replicated).
    # cb16 [128, 52+128] bf16: cqkT swizzle | 128x128 identity
    d_cb16 = nc.dram_tensor("cb16", [128, JT * R + 128], bf16, kind="ExternalInput")
    # cbt [4, 130] bf16: B.T one-hot | label | c2 (r*K)
    d_cbt = nc.dram_tensor("cbt", [R, 130], bf16, kind="ExternalInput")
    # cf32 [128, 3] f32: c3 | ck_mask as 0/1 | NEGINF
    d_cf32 = nc.dram_tensor("cf32", [128, 3], f32, kind="ExternalInput")
    d_wcqkT = nc.dram_tensor("wcqkT", [JPAD, H], bf16, kind="ExternalInput")
    d_wk = nc.dram_tensor("wk", [H, HP], bf16, kind="ExternalInput")
    d_pe1x = nc.dram_tensor("pe1x", [R * K, HP], f32, kind="ExternalInput")
    d_pe1 = nc.dram_tensor("pe1", [R * K, H], f32, kind="ExternalInput")
    d_pe0 = nc.dram_tensor("pe0", [GROWS, GCOLS], f32, kind="ExternalInput")
    d_ptok = nc.dram_tensor("ptok", [R * K, tokc], i32, kind="ExternalInput")
    d_pmask = nc.dram_tensor("pmask", [R * K, T], u8, kind="ExternalInput")

    # Outputs.
    o_score = nc.dram_tensor("score", [R * K, 1], f32, kind="ExternalOutput")
    o_senc = nc.dram_tensor("senc", [R * T * H // GCOLS, GCOLS], f32,
                            kind="ExternalOutput")
    o_smask = nc.dram_tensor("smask", [R, T], u8, kind="ExternalOutput")
    o_suse = nc.dram_tensor("suse", [R, H], f32, kind="ExternalOutput")
    o_sidx = nc.dram_tensor("sidx", [R, tokc], i32, kind="ExternalOutput")

    add = mybir.AluOpType.add
    mult = mybir.AluOpType.mult
    IDC = JT * R               # identity starts at this column of cb16

    with tile.TileContext(nc) as tc:
        with (
            tc.tile_pool(name="sb", bufs=1) as sb,
            tc.tile_pool(name="ps", bufs=1, space="PSUM") as ps,
        ):
            # ---- constants (tiny, early)
            cb16_sb = sb.tile([128, JT * R + 128], bf16)
            nc.scalar.dma_start(cb16_sb[:], d_cb16.ap())
            cbt_sb = sb.tile([R, 130], bf16)
            nc.scalar.dma_start(cbt_sb[:], d_cbt.ap())
            cf32_sb = sb.tile([128, 3], f32)
            nc.scalar.dma_start(cf32_sb[:], d_cf32.ap())

            # ---- weight streams (sync HWDGE queue)
            wc_chunks = []
            off = 0
            for ci, ntile in enumerate(WC_CHUNKS):
                w_c = sb.tile([128, ntile * H], bf16, tag=f"wc{ci}")
                nc.sync.dma_start(
                    w_c[:],
                    d_wcqkT.ap()[off * 128:(off + ntile) * 128, :].rearrange(
                        "(t p) h -> p t h", p=128))
                wc_chunks.append((off, ntile, w_c))
                off += ntile
            wk_chunks = []
            off = 0
            for ci, ntile in enumerate(WK_CHUNKS):
                wk_c = sb.tile([128, ntile * HP], bf16, tag=f"wk{ci}")
                nc.sync.dma_start(
                    wk_c[:],
                    d_wk.ap()[off * 128:(off + ntile) * 128, :].rearrange(
                        "(s p) h -> p s h", p=128))
                wk_chunks.append((off, ntile, wk_c))
                off += ntile

            pe1_sb = sb.tile([R * K, HP], f32)
            nc.scalar.dma_start(pe1_sb[:], d_pe1x.ap())

            # ---- gather index computation (from label input, on device)
            lbl_bf = cbt_sb[:, 128:129]
            c2_bf = cbt_sb[:, 129:130]
            idx2f = sb.tile([R, 1], bf16)
            nc.vector.tensor_tensor(out=idx2f[:], in0=lbl_bf, in1=c2_bf, op=add)
            idx2 = sb.tile([R, 1], i32)
            nc.vector.tensor_copy(idx2[:], idx2f[:])
            p_lbl = ps.tile([128, 1], f32, tag="tiny", bufs=2)
            nc.tensor.matmul(p_lbl[:], lhsT=cbt_sb[:, :128], rhs=lbl_bf,
                             start=True, stop=True)
            idx3f = sb.tile([128, 1], f32)
            nc.vector.tensor_scalar(out=idx3f[:], in0=p_lbl[:], scalar1=32.0,
                                    scalar2=None, op0=mult)
            nc.vector.tensor_tensor(out=idx3f[:], in0=idx3f[:],
                                    in1=cf32_sb[:, 0:1], op=add)
            idx3 = sb.tile([128, 1], i32)
            nc.vector.tensor_copy(idx3[:], idx3f[:])

            # ---- indirect gathers (bit-exact copies; overlap weight stream)
            g0 = sb.tile([128, GCOLS], f32)
            nc.gpsimd.indirect_dma_start(
                out=g0[:], out_offset=None, in_=d_pe0.ap(),
                in_offset=IndirectOffsetOnAxis(ap=idx3[:, :1], axis=0))
            nc.scalar.dma_start(o_senc.ap(), g0[:])
            g1 = sb.tile([R, H], f32)
            nc.gpsimd.indirect_dma_start(
                out=g1[:], out_offset=None, in_=d_pe1.ap(),
                in_offset=IndirectOffsetOnAxis(ap=idx2[:, :1], axis=0))
            nc.scalar.dma_start(o_suse.ap(), g1[:])
            g2 = sb.tile([R, T], u8)
            nc.gpsimd.indirect_dma_start(
                out=g2[:], out_offset=None, in_=d_pmask.ap(),
                in_offset=IndirectOffsetOnAxis(ap=idx2[:, :1], axis=0))
            nc.scalar.dma_start(o_smask.ap(), g2[:])
            g3 = sb.tile([R, tokc], i32)
            nc.gpsimd.indirect_dma_start(
                out=g3[:], out_offset=None, in_=d_ptok.ap(),
                in_offset=IndirectOffsetOnAxis(ap=idx2[:, :1], axis=0))
            nc.scalar.dma_start(o_sidx.ap(), g3[:])

            # ---- stage 1: A[r,g] = sum_j cqk_ext[r,j] * wcqkT[j,g]
            pA = ps.tile([R, H], f32, tag="acc")
            for lo, hi in ((0, 512), (512, H)):
                for off, ntile, w_c in wc_chunks:
                    for tl in range(ntile):
                        t = off + tl
                        nc.tensor.matmul(
                            pA[:, lo:hi],
                            lhsT=cb16_sb[:, t * R:(t + 1) * R],
                            rhs=w_c[:, tl * H + lo:tl * H + hi],
                            start=(t == 0), stop=(t == JT - 1))
            A_sb = sb.tile([R, H], bf16)
            nc.vector.tensor_copy(A_sb[:], pA[:])

            # ---- transpose A -> AT (PE transpose, 128-col blocks)
            AT_sb = sb.tile([128, ST * R], bf16)
            for s in range(ST):
                pT = ps.tile([128, R], bf16, tag="tiny", bufs=2)
                nc.tensor.transpose(pT[:], A_sb[:, s * 128:(s + 1) * 128],
                                    cb16_sb[:R, IDC:IDC + R])
                nc.vector.tensor_copy(AT_sb[:, s * R:(s + 1) * R], pT[:])

            # ---- stage 2: u_ext[r,h] = sum_g A[r,g] * wk_ext[g,h]
            pU = ps.tile([R, HP], f32, tag="acc")
            for lo, hi in ((0, 512), (512, HP)):
                for off, ntile, wk_c in wk_chunks:
                    for sl in range(ntile):
                        s = off + sl
                        nc.tensor.matmul(
                            pU[:, lo:hi],
                            lhsT=AT_sb[:, s * R:(s + 1) * R],
                            rhs=wk_c[:, sl * HP + lo:sl * HP + hi],
                            start=(s == 0), stop=(s == ST - 1))
            u_sb = sb.tile([R, HP], bf16)
            nc.vector.tensor_copy(u_sb[:], pU[:])

            # ---- broadcast u rows to 32 partitions each: uexp[p,:] = u[p//32,:]
            pUE = ps.tile([128, HP], f32, tag="uexp")
            for lo, hi in ((0, 512), (512, HP)):
                nc.tensor.matmul(pUE[:, lo:hi], lhsT=cbt_sb[:, :128],
                                 rhs=u_sb[:, lo:hi], start=True, stop=True)

            # ---- score: rowwise dot + ck_mask select
            prod = sb.tile([R * K, HP], f32)
            nc.vector.tensor_mul(prod[:], pe1_sb[:], pUE[:])
            sc = sb.tile([R * K, 1], f32)
            nc.vector.tensor_reduce(sc[:], prod[:], mybir.AxisListType.X, add)
            cku8 = sb.tile([R * K, 1], u8)
            nc.vector.tensor_copy(cku8[:], cf32_sb[:, 1:2])
            scm = sb.tile([R * K, 1], f32)
            nc.vector.select(scm[:], cku8[:], sc[:], cf32_sb[:, 2:3])
            nc.scalar.dma_start(o_score.ap(), scm[:])

    nc.compile()
    return nc


def _host_prep(inputs):
    ctx = np.asarray(inputs["contexts_encoded_1"], np.float32)
    tku = np.asarray(inputs["tracked_knowledge_use"], np.float32)
    pe0 = np.ascontiguousarray(np.asarray(inputs["pool_encoded_0"], np.float32))
    pe1 = np.ascontiguousarray(np.asarray(inputs["pool_encoded_1"], np.float32))
    pm = np.ascontiguousarray(np.asarray(inputs["pool_mask"]))
    ckm = np.ascontiguousarray(np.asarray(inputs["ck_mask"]))
    lbl = np.asarray(inputs["label"]).astype(np.int32)
    ptok = np.ascontiguousarray(np.asarray(inputs["pool_tokens"]))
    Wc = np.asarray(inputs["W_cqk"], np.float32)
    bc = np.asarray(inputs["b_cqk"], np.float32)
    Wk = np.asarray(inputs["W_k"], np.float32)
    bk = np.asarray(inputs["b_k"], np.float32)

    tok_itemsize = ptok.dtype.itemsize
    assert tok_itemsize % 4 == 0
    tokc = T * (tok_itemsize // 4)

    wcqkT = np.zeros((JPAD, H), np.float32)
    wcqkT[:2 * H] = Wc.T
    wcqkT[2 * H] = bc
    wcqkT = wcqkT.astype(bfnp)
    wk_ext = np.concatenate([Wk, bk[:, None]], axis=1).astype(bfnp)

    p = np.arange(128)
    bt = np.zeros((R, 128), np.float32)
    bt[p // K, p] = 1.0
    c3 = ((p // K) * (K * 32) + (p % 32)).astype(np.float32)
    ident = np.eye(128, dtype=np.float32)

    in_maps = []
    for c in range(NCORES):
        sl = slice(c * R, (c + 1) * R)
        cqk_ext = np.zeros((R, JPAD), np.float32)
        cqk_ext[:, :H] = ctx[sl, 2, :]
        cqk_ext[:, H:2 * H] = tku[sl]
        cqk_ext[:, 2 * H] = 1.0
        cqkT_sw = np.ascontiguousarray(
            cqk_ext.reshape(R, JT, 128).transpose(2, 1, 0)).reshape(128, JT * R)
        cb16 = np.concatenate([cqkT_sw, ident], axis=1).astype(bfnp)
        cbt = np.concatenate(
            [bt, lbl[sl].astype(np.float32)[:, None],
             (np.arange(R, dtype=np.float32) * K)[:, None]], axis=1).astype(bfnp)
        cf32 = np.stack(
            [c3, ckm[sl].reshape(R * K).astype(np.float32),
             np.full(128, NEGINF, np.float32)], axis=1)
        pe1c = pe1[sl].reshape(R * K, H)
        pe1x = np.concatenate(
            [pe1c, np.ones((R * K, 1), np.float32)], axis=1)
        in_maps.append({
            "cb16": cb16,
            "cbt": cbt,
            "cf32": np.ascontiguousarray(cf32),
            "wcqkT": wcqkT,
            "wk": wk_ext,
            "pe1x": pe1x,
            "pe1": np.ascontiguousarray(pe1c),
            "pe0": pe0[sl].reshape(GROWS, GCOLS),
            "ptok": np.ascontiguousarray(ptok[sl]).view(np.int32).reshape(R * K, tokc),
            "pmask": pm[sl].reshape(R * K, T).view(np.uint8),
        })
    return in_maps, tokc, ptok.dtype, np.asarray(inputs["pool_mask"]).dtype


def _run(inputs, trace=False, trace_cores=None):
    in_maps, tokc, tok_dtype, mask_dtype = _host_prep(inputs)
    if tokc not in _CACHE:
        _CACHE[tokc] = _build_nc(tokc)
    nc = _CACHE[tokc]
    res = run_bass_kernel_spmd(
        nc, in_maps, core_ids=list(range(NCORES)),
        trace=trace, trace_cores=trace_cores,
    )
    score = np.concatenate(
        [res.results[c]["score"].reshape(R, K) for c in range(NCORES)], axis=0)
    senc = np.concatenate(
        [res.results[c]["senc"].reshape(R, T, H) for c in range(NCORES)], axis=0)
    smask = np.concatenate(
        [res.results[c]["smask"].reshape(R, T) for c in range(NCORES)],
        axis=0).view(mask_dtype)
    suse = np.concatenate(
        [res.results[c]["suse"].reshape(R, H) for c in range(NCORES)], axis=0)
    sidx = np.concatenate(
        [res.results[c]["sidx"].reshape(R, tokc) for c in range(NCORES)],
        axis=0).view(tok_dtype).reshape(N, T)
    return (score, senc, smask, suse, sidx), res


def kernel(**inputs):
    outs, _ = _run(inputs, trace=False)
    return outs
